# revision 58
# baseline (speedup 1.0000x reference)
"""Fused Attention1d block (groupnorm -> qkv conv1x1 -> attention -> groupnorm
-> proj conv1x1 -> residual) for Trainium2, data-parallel over batch: 8 batch
elements -> 8 NeuronCores, no collectives.

Per-core layout (x_b is [C=512, T=2048], channels on partitions):
  - gn stats: per-channel bn_stats (fp32), group-combine via tiny fp32 PE
    matmuls with a 0/1 selector, expand back with the transposed selector.
    Affine applies are split across ACT / DVE / GPSIMD.
  - qkv matmul computes only Q,K in a packed head-pair layout (pair p tile has
    head 2p on partitions 0:64, head 2p+1 on 64:128).  V is produced directly
    TRANSPOSED (V^T [t, ch]) by a separate matmul, with a ones-column appended
    per head (weight column of zeros + bias 1.0).
  - attention is head-PAIRED: the two K=64 score matmuls of a pair are issued
    back-to-back with lhsT/rhs base partitions 0 and 64, which auto-derives
    tile_position (0,0)/(64,0) so they run CONCURRENTLY in disjoint row
    groups of the PE array (the K=64 half-array waste cancels out).
  - softmax: P = exp(S/8) unnormalized (S is O(+-6), max-subtract
    unnecessary).  The exp is split per-tile between the ACT engine (true
    exp) and the DVE (Schraudolph: bf16 bits = rint(x*128/ln2 + 16248.5)
    written through an int16 bitcast; ~1.8% rms per-element error that
    cancels to <0.1% after the softmax average).  Each s-chunk sends one head
    to ACT and one to DVE so neither engine is the latency bottleneck.
  - emission is software-pipelined one s-chunk ahead (scores of chunk i+1
    issue before AV of chunk i) so the in-order PE queue never stalls on exp.
  - softmax denominators fall out of the AV matmul for free via a constant
    ones-column in V^T (row 64 of the AV accumulator); the ones columns are
    memset into the V^T tiles rather than carried as matmul columns.
  - AV psum is evicted with one ACT copy per head into an SBUF staging tile
    (DMA cannot read PSUM), then fanned out by DMA: rows 0:64 to h, row 64
    (the denominator) to l.
  - normalization after AV: 1/l via ACT ln/exp (bf16), partition-broadcast
    ON THE PE (a ones[1,64] stationary column replicates an r row chunk to
    64 partitions per matmul; the DVE multiply reads it straight from PSUM).
    This replaced a DRAM-bounce broadcast that cost ~40us of DMA latency.
  - x stays resident in SBUF for the residual; proj weights load up front.
  - all large matmuls run with bf16 operands (full PE rate, fp32
    accumulate); statistics and softmax bookkeeping stay fp32.
"""

import numpy as np
import ml_dtypes

import concourse.bass as bass
import concourse.tile as tile
from concourse import bacc, mybir
from concourse.bass_utils import run_bass_kernel_spmd

AF = mybir.ActivationFunctionType
ALU = mybir.AluOpType
F32 = mybir.dt.float32
F32R = mybir.dt.float32r
BF16 = mybir.dt.bfloat16
I16 = mybir.dt.int16

NCORES = 8
B, C, T = 8, 512, 2048
H = 8            # attention heads
CH = 64          # channels per head
G = 32           # groupnorm groups
GS = C // G      # 16 channels per group
EPS = 1e-5
KC = C // 128    # 4 channel chunks
TC5 = T // 512   # 4 t-chunks of 512
SC = T // 128    # 16 s-chunks of 128

# q and k are each scaled by 1/sqrt(sqrt(CH)); apply the squared scale once
# inside the exp.
_s = np.float32(1.0) / np.sqrt(np.sqrt(np.float32(CH)))
SCALE2 = float(np.float32(_s) * np.float32(_s))
# Schraudolph exp in bf16 bits: bits = rint(x * 128/ln2 + (127*128 - 7.5))
SCHR_A = float(128.0 / np.log(2.0)) * SCALE2
SCHR_B = 16248.5

_CACHE = {}


def _bcast_rows(src_row, nrows):
    """AP that reads one [1, N] sbuf row nrows times (partition broadcast)."""
    return bass.AP(tensor=src_row.tensor, offset=src_row.offset,
                   ap=[[0, nrows], list(src_row.ap[-1])])


def _groupnorm(nc, stats, gnps, src, dst, sel_sb, selt_sb, gb_sb, gcol, eps32):
    """Group norm over 4 channel-chunk tiles. src: 4 fp32 [128,>=T] APs,
    dst: 4 [128,>=T] APs (any dtype). gb_sb [128,16]: gamma cols gcol..gcol+3,
    beta cols gcol+4..gcol+7."""
    rs_list = []
    for k in range(KC):
        st = stats.tile([128, 4, 6], F32, tag="bnst")
        for sub in range(4):
            nc.vector.bn_stats(out=st[:, sub, :],
                               in_=src[k][:, 512 * sub:512 * (sub + 1)])
        mv = stats.tile([128, 2], F32, tag="bnmv")
        nc.vector.bn_aggr(out=mv, in_=st)
        # rs = [mean, E[x^2]] per channel
        rs = stats.tile([128, 2], F32, tag="bnrs")
        nc.vector.tensor_mul(out=rs[:, 1:2], in0=mv[:, 0:1], in1=mv[:, 0:1])
        nc.vector.tensor_add(out=rs[:, 1:2], in0=rs[:, 1:2], in1=mv[:, 1:2])
        nc.vector.tensor_copy(out=rs[:, 0:1], in_=mv[:, 0:1])
        rs_list.append(rs)

    # group stats [G,2] = sum_k sel_k.T @ rs_k -> (mean_g, E2_g)
    gp = gnps.tile([G, 2], F32, tag="gps")
    for k in range(KC):
        nc.tensor.matmul(gp, lhsT=sel_sb[:, k, :], rhs=rs_list[k],
                         start=(k == 0), stop=(k == KC - 1))
    # gg rows 0:G = [mean_g, rstd_g]; rows G:128 zero
    gg = stats.tile([128, 2], F32, tag="gng")
    # partition regions starting at 32 may span at most 32 partitions
    nc.vector.memset(gg[32:64, :], 0.0)
    nc.vector.memset(gg[64:128, :], 0.0)
    nc.vector.tensor_copy(out=gg[:G, 0:1], in_=gp[:, 0:1])
    tmp = stats.tile([G, 1], F32, tag="gnt")
    # square the mean from its SBUF copy (two PSUM operands in one DVE op are
    # rejected by the BIR verifier)
    nc.vector.tensor_mul(out=tmp, in0=gg[:G, 0:1], in1=gg[:G, 0:1])
    nc.vector.tensor_tensor(out=gg[:G, 1:2], in0=gp[:, 1:2], in1=tmp,
                            op=ALU.subtract)
    # rsqrt(v+eps) = exp(-0.5*ln(v+eps)): keeps every activation in the
    # kernel inside the single natural_log_exp ACT table set.
    nc.scalar.activation(out=gg[:G, 1:2], in_=gg[:G, 1:2], func=AF.Ln,
                         bias=eps32, scale=1.0)
    nc.scalar.activation(out=gg[:G, 1:2], in_=gg[:G, 1:2], func=AF.Exp,
                         scale=-0.5)

    for k in range(KC):
        ex = gnps.tile([128, 2], F32, tag="gex")
        nc.tensor.matmul(ex, lhsT=selt_sb[:, 128 * k:128 * (k + 1)], rhs=gg,
                         start=True, stop=True)
        # A = rstd*gamma ; Bc = beta - mean*A ; out = x*A + Bc
        ab = stats.tile([128, 2], F32, tag="gnab")
        nc.vector.tensor_mul(out=ab[:, 0:1], in0=ex[:, 1:2],
                             in1=gb_sb[:, gcol + k:gcol + k + 1])
        nc.vector.tensor_mul(out=ab[:, 1:2], in0=ex[:, 0:1], in1=ab[:, 0:1])
        nc.vector.tensor_tensor(out=ab[:, 1:2],
                                in0=gb_sb[:, gcol + 4 + k:gcol + 5 + k],
                                in1=ab[:, 1:2], op=ALU.subtract)
        # spread the big applies: 2 chunks ACT, 1 DVE, 1 GPSIMD
        if k < 2:
            nc.scalar.activation(out=dst[k][:, 0:T], in_=src[k][:, 0:T],
                                 func=AF.Identity, bias=ab[:, 1:2],
                                 scale=ab[:, 0:1])
        elif k == 2:
            nc.vector.tensor_scalar(out=dst[k][:, 0:T], in0=src[k][:, 0:T],
                                    scalar1=ab[:, 0:1], scalar2=ab[:, 1:2],
                                    op0=ALU.mult, op1=ALU.add)
        else:
            nc.gpsimd.tensor_scalar(out=dst[k][:, 0:T], in0=src[k][:, 0:T],
                                    scalar1=ab[:, 0:1], scalar2=ab[:, 1:2],
                                    op0=ALU.mult, op1=ALU.add)


def _kernel_body(nc, tc, d, out_d, reps=1, stop_after="F"):
    if reps > 1:
        with tc.For_i(0, reps, 1):
            _kernel_body_inner(nc, tc, d, out_d, stop_after)
    else:
        _kernel_body_inner(nc, tc, d, out_d, stop_after)


def _kernel_body_inner(nc, tc, d, out_d, stop_after="F"):
    import contextlib
    ctx = contextlib.ExitStack()
    with ctx:
        # ---- persistent SBUF pools ----
        big4 = ctx.enter_context(tc.tile_pool(name="big4", bufs=4))
        act4 = ctx.enter_context(tc.tile_pool(name="act4", bufs=7))
        qkp = ctx.enter_context(tc.tile_pool(name="qkp", bufs=8))
        wqkp = ctx.enter_context(tc.tile_pool(name="wqkp", bufs=4))
        wvp = ctx.enter_context(tc.tile_pool(name="wvp", bufs=4))
        ptp = ctx.enter_context(tc.tile_pool(name="ptp", bufs=8))
        stgp = ctx.enter_context(tc.tile_pool(name="stgp", bufs=4))
        vtp = ctx.enter_context(tc.tile_pool(name="vtp", bufs=4))
        wpp = ctx.enter_context(tc.tile_pool(name="wpp", bufs=4))
        small = ctx.enter_context(tc.tile_pool(name="small", bufs=1))
        stats = ctx.enter_context(tc.tile_pool(name="stats", bufs=4))
        lrp = ctx.enter_context(tc.tile_pool(name="lrp", bufs=1))
        rbp = ctx.enter_context(tc.tile_pool(name="rbp", bufs=3))
        outp = ctx.enter_context(tc.tile_pool(name="outp", bufs=4))

        # ---- load x first: it heads the critical path (gn1 stats).
        # Issue the four chunk DMAs from four different queues so their
        # descriptor generation runs in parallel.
        xt = [big4.tile([128, 2080], F32, tag="big", name=f"xt{i}")
              for i in range(KC)]
        for k, eng in zip(range(KC), (nc.sync, nc.scalar, nc.gpsimd,
                                      nc.sync)):
            eng.dma_start(xt[k][:, 0:T], d["x"][128 * k:128 * (k + 1), :])

        # ---- constants / weights ----
        sel_sb = small.tile([128, KC, G], F32)
        nc.sync.dma_start(sel_sb, d["sel"].rearrange("(k p) g -> p k g", p=128))
        selt_sb = small.tile([128, C], F32)
        nc.sync.dma_start(selt_sb, d["selt"])
        gb_sb = small.tile([128, 16], F32)  # g1[0:4] b1[4:8] g2[8:12] b2[12:16]
        nc.sync.dma_start(gb_sb[:, 0:4], d["g1"].rearrange("(k p) -> p k", p=128))
        nc.sync.dma_start(gb_sb[:, 4:8], d["b1"].rearrange("(k p) -> p k", p=128))
        nc.sync.dma_start(gb_sb[:, 8:12], d["g2"].rearrange("(k p) -> p k", p=128))
        nc.sync.dma_start(gb_sb[:, 12:16], d["b2"].rearrange("(k p) -> p k", p=128))
        bqk_sb = small.tile([128, H], F32)
        nc.sync.dma_start(bqk_sb, d["bqk"].rearrange("(m p) -> p m", p=128))
        bp_sb = small.tile([128, KC], F32)
        nc.sync.dma_start(bp_sb, d["bp"].rearrange("(m p) -> p m", p=128))
        bv_sb = small.tile([128, 512], F32)
        bv_bcast = bass.AP(tensor=d["bv"].tensor, offset=d["bv"].offset,
                           ap=[[0, 128]] + [list(a) for a in d["bv"].ap])
        nc.sync.dma_start(bv_sb, bv_bcast)
        eps32 = small.tile([G, 1], F32)
        nc.vector.memset(eps32, EPS)

        wqk_sb = [wqkp.tile([128, 1024], BF16, tag="wqkpt", name=f"wqk{i}")
                  for i in range(KC)]
        for k in range(KC):
            nc.sync.dma_start(wqk_sb[k], d["wqk"][128 * k:128 * (k + 1), :])
        wv_sb = [wvp.tile([128, 512], BF16, tag="wvwp", name=f"wv{i}")
                 for i in range(KC)]
        for k in range(KC):
            nc.sync.dma_start(wv_sb[k], d["wv"][128 * k:128 * (k + 1), :])
        wp_sb = [wpp.tile([128, 512], BF16, tag="wp", name=f"wp{i}")
                 for i in range(KC)]
        for k in range(KC):
            nc.sync.dma_start(wp_sb[k], d["wp"][128 * k:128 * (k + 1), :])

        xn = [act4.tile([128, T], BF16, tag="act", name=f"xn{i}")
              for i in range(KC)]

        # ---- phase A: gn1 (x -> xn, bf16) ----
        with tc.tile_pool(name="gnps1", bufs=2, space="PSUM") as gnps:
            _groupnorm(nc, stats, gnps, xt, xn, sel_sb, selt_sb, gb_sb, 0, eps32)

        if stop_after == "A":
            return
        # ---- phase B: packed Q/K matmul + V^T matmul ----
        qk_sb = [qkp.tile([128, T], BF16, tag="qk", name=f"qk{i}")
                 for i in range(H)]
        # V^T gets its own pool so the xt tiles stay resident for the
        # residual add (no x reload before proj).
        vt_sb = [vtp.tile([128, 2080], BF16, tag="vt", name=f"vt{i}")
                 for i in range(4)]
        with tc.tile_pool(name="mmps", bufs=4, space="PSUM") as mmps:
            # contraction split into K=64 halves as TWO CLEAN accumulation
            # groups (separate psum tiles, each with its own start/stop) on
            # disjoint PE row groups, MMs interleaved pairwise: the halves
            # run concurrently and each LDWEIGHTS prefetches under the other
            # half's matmul (serial same-row LDW costs ~107ns/MM).
            for mt in range(H):
                for n in range(TC5):
                    ps_e = mmps.tile([128, 512], F32, tag="mmqke", bufs=2,
                                     name="psqke")
                    ps_o = mmps.tile([128, 512], F32, tag="mmqko", bufs=2,
                                     name="psqko")
                    for k in range(KC):
                        nc.tensor.matmul(
                            ps_e, lhsT=wqk_sb[k][0:64, 128 * mt:128 * (mt + 1)],
                            rhs=xn[k][0:64, 512 * n:512 * (n + 1)],
                            start=(k == 0), stop=(k == KC - 1))
                        nc.tensor.matmul(
                            ps_o, lhsT=wqk_sb[k][64:128, 128 * mt:128 * (mt + 1)],
                            rhs=xn[k][64:128, 512 * n:512 * (n + 1)],
                            start=(k == 0), stop=(k == KC - 1))
                    # evict in two ops (the verifier rejects two PSUM
                    # operands in one DVE op): ACT adds bias to the even
                    # half, DVE adds the odd half in place.
                    dst = qk_sb[mt][:, 512 * n:512 * (n + 1)]
                    nc.scalar.activation(out=dst, in_=ps_e, func=AF.Identity,
                                         bias=bqk_sb[:, mt:mt + 1],
                                         scale=1.0)
                    nc.vector.tensor_add(out=dst, in0=dst, in1=ps_o)
            # ones columns of V^T (the softmax-denominator trick) are a
            # constant: write them once per tile with a strided memset
            # instead of carrying 8 extra matmul columns.
            for t in range(4):
                ones_view = vt_sb[t][:, 0:2080].rearrange(
                    "p (b h c) -> p b h c", b=4, c=65)[:, :, :, 64:65]
                nc.vector.memset(ones_view, 1.0)
            for bsc in range(SC):
                # V^T: t-chunk b -> vt_sb[b//4][:, 520*(b%4) : 520*(b%4)+520]
                # (per head: 64 v columns + the ones column at offset 64)
                ps = mmps.tile([128, 512], F32, tag="mmvt", bufs=2, name="psvt")
                for k in range(KC):
                    lhsT = xn[k][:, 128 * bsc:128 * (bsc + 1)]
                    nc.tensor.matmul(ps, lhsT=lhsT, rhs=wv_sb[k],
                                     start=(k == 0), stop=(k == KC - 1))
                c0 = 520 * (bsc % 4)
                out_view = vt_sb[bsc // 4][:, c0:c0 + 520].rearrange(
                    "p (h c) -> p h c", c=65)[:, :, 0:64]
                nc.vector.tensor_add(
                    out=out_view,
                    in0=ps.rearrange("p (h c) -> p h c", c=64),
                    in1=bv_sb.rearrange("p (h c) -> p h c", c=64))

        if stop_after == "B":
            return
        # ---- phase C: attention, head-paired, software-pipelined ----
        h_sb = [act4.tile([128, T], F32, tag="act", name=f"hsb{i}")
                for i in range(H // 2)]
        l_sb = lrp.tile([8, T], F32, tag="lsb")

        units = [(p2, tblk) for p2 in range(H // 2) for tblk in range(TC5)]
        av_tiles = {}

        with tc.tile_pool(name="scps", bufs=4, space="PSUM") as scps, \
             tc.tile_pool(name="avps", bufs=4, space="PSUM") as avps:

            def emit_sc(p2, tblk, bsc):
                qt = qk_sb[2 * p2]
                kt = qk_sb[2 * p2 + 1]
                t0 = 512 * tblk
                sc_a = scps.tile([128, 512], F32, tag="sc", bufs=4, name="sca")
                sc_b = scps.tile([128, 512], F32, tag="sc", bufs=4, name="scb")
                s0 = 128 * bsc
                # base partitions 0 / 64 auto-derive tile_position (0,0) and
                # (64,0): the two matmuls run concurrently in disjoint row
                # groups of the PE array (HW-verified 2.85x vs same-row).
                nc.tensor.matmul(sc_a, lhsT=kt[0:64, s0:s0 + 128],
                                 rhs=qt[0:64, t0:t0 + 512],
                                 start=True, stop=True)
                nc.tensor.matmul(sc_b, lhsT=kt[64:128, s0:s0 + 128],
                                 rhs=qt[64:128, t0:t0 + 512],
                                 start=True, stop=True)
                return sc_a, sc_b

            def get_av(p2, tblk):
                if (p2, tblk) not in av_tiles:
                    av_tiles[(p2, tblk)] = (
                        avps.tile([65, 512], F32, tag="av", bufs=4,
                                  name="ava"),
                        avps.tile([65, 512], F32, tag="av", bufs=4,
                                  name="avb"))
                return av_tiles[(p2, tblk)]

            def emit_evict(p2, tblk):
                # evict to SBUF staging with one ACT copy per head (DMA
                # cannot read PSUM), then fan out by DMA: rows 0:64 to
                # h_sb, row 64 (the softmax denominator) to l_sb.
                t0 = 512 * tblk
                av_a, av_b = av_tiles[(p2, tblk)]
                for i, (av, r0) in enumerate(((av_a, 0), (av_b, 64))):
                    stg = stgp.tile([65, 512], F32, tag="stg", bufs=4,
                                    name="stg")
                    nc.scalar.activation(out=stg, in_=av, func=AF.Copy)
                    nc.sync.dma_start(
                        h_sb[p2][r0:r0 + 64, t0:t0 + 512], stg[0:64, :])
                    hh = 2 * p2 + i
                    nc.sync.dma_start(
                        l_sb[hh:hh + 1, t0:t0 + 512], stg[64:65, :])

            def emit_expav2(p2, tA, tB, bsc, scA_a, scA_b, scB_a, scB_b):
                """exp + AV for s-chunk `bsc` of BOTH interleaved t-block
                units.  Interleaving two units doubles the exp latency
                window (~1.5us vs ~0.85) at identical PSUM cost, and lets
                the two AVs of each head share one stationary V slice."""
                avA_a, avA_b = get_av(p2, tA)
                avB_a, avB_b = get_av(p2, tB)
                pts = {}
                for key, sch in (("A_a", scA_a), ("A_b", scA_b),
                                 ("B_a", scB_a), ("B_b", scB_b)):
                    pts[key] = ptp.tile([128, 512], BF16, tag="pt", bufs=8,
                                        name="pt" + key)
                # even engine split: ACT's staging copies already offset its
                # faster per-tile exp (ACT 570ns vs DVE 658)
                act_keys = ("A_a", "B_b") if bsc % 2 == 0 else ("A_b", "B_a")
                for key, sch in (("A_a", scA_a), ("A_b", scA_b),
                                 ("B_a", scB_a), ("B_b", scB_b)):
                    if key in act_keys:
                        nc.scalar.activation(out=pts[key], in_=sch,
                                             func=AF.Exp, scale=SCALE2)
                    else:
                        nc.vector.tensor_scalar(
                            out=pts[key].bitcast(I16), in0=sch,
                            scalar1=SCHR_A, scalar2=SCHR_B,
                            op0=ALU.mult, op1=ALU.add)
                # AVs grouped by stationary slice (same lhsT back-to-back)
                for h, jobs in ((2 * p2, ((avA_a, pts["A_a"]),
                                          (avB_a, pts["B_a"]))),
                                (2 * p2 + 1, ((avA_b, pts["A_b"]),
                                              (avB_b, pts["B_b"])))):
                    vslice = vt_sb[bsc // 4][:, 520 * (bsc % 4) + 65 * h:
                                             520 * (bsc % 4) + 65 * (h + 1)]
                    for av, pt in jobs:
                        nc.tensor.matmul(av, lhsT=vslice, rhs=pt,
                                         start=(bsc == 0),
                                         stop=(bsc == SC - 1))
                if bsc == SC - 1:
                    emit_evict(p2, tA)
                    emit_evict(p2, tB)

            # one flat software-pipelined stream: pending carries ACROSS
            # unit-pair boundaries so the pipeline never drains (each reset
            # cost ~1.5-2.5us of PE bubble at the 8 boundaries)
            pending = None
            for p2 in range(H // 2):
                for tA, tB in ((0, 1), (2, 3)):
                    for bsc in range(SC):
                        scA = emit_sc(p2, tA, bsc)
                        scB = emit_sc(p2, tB, bsc)
                        if pending is not None:
                            emit_expav2(*pending)
                        pending = (p2, tA, tB, bsc, *scA, *scB)
            emit_expav2(*pending)

        if stop_after == "C":
            return
        # ---- phase D: normalize by 1/l ----
        # 1/l as exp(-ln(l)): l is a sum of positives in [~5e2, ~5e3]; both
        # functions live in one ACT table set.
        nc.scalar.activation(out=l_sb, in_=l_sb, func=AF.Ln)
        # partition-broadcast of the r rows on the PE (idle here): a ones
        # [1,64] bf16 stationary column broadcasts a [1,512] r chunk to 64
        # partitions in one matmul (full rate at N=512).  The DVE mul then
        # reads the broadcast straight from PSUM -- no DRAM bounce.  r in
        # bf16 adds ~0.1% gain jitter, well inside budget.
        r_bf = lrp.tile([8, T], BF16, tag="rbf")
        nc.scalar.activation(out=r_bf, in_=l_sb, func=AF.Exp, scale=-1.0)
        ones64 = small.tile([1, 64], BF16)
        nc.vector.memset(ones64, 1.0)
        # gn2 stats are fused per pair: chunk p2's bn_stats depend only on
        # pair p2's normalize, so they issue right after its muls instead of
        # waiting for the whole phase (the serial D->E boundary measured
        # ~25us of dead time).
        rs_list = []
        with tc.tile_pool(name="rbps", bufs=4, space="PSUM") as rbps:
            for p2 in range(H // 2):
                rp_a = lrp.tile([1, T], BF16, tag="rp", bufs=2, name="rpa")
                rp_b = lrp.tile([1, T], BF16, tag="rp", bufs=2, name="rpb")
                nc.sync.dma_start(rp_a, r_bf[2 * p2:2 * p2 + 1, :])
                nc.sync.dma_start(rp_b, r_bf[2 * p2 + 1:2 * p2 + 2, :])
                for n in range(TC5):
                    rbb = rbps.tile([128, 512], F32, tag="rb", bufs=4)
                    for i, rp in enumerate((rp_a, rp_b)):
                        nc.tensor.matmul(
                            rbb[64 * i:64 * i + 64, :],
                            lhsT=ones64,
                            rhs=rp[:, 512 * n:512 * (n + 1)],
                            start=True, stop=True)
                    nc.vector.tensor_mul(
                        out=h_sb[p2][:, 512 * n:512 * (n + 1)],
                        in0=h_sb[p2][:, 512 * n:512 * (n + 1)],
                        in1=rbb)
                st = stats.tile([128, 4, 6], F32, tag="bnst")
                for sub in range(4):
                    nc.vector.bn_stats(out=st[:, sub, :],
                                       in_=h_sb[p2][:, 512 * sub:512 * (sub + 1)])
                mv = stats.tile([128, 2], F32, tag="bnmv")
                nc.vector.bn_aggr(out=mv, in_=st)
                rs = stats.tile([128, 2], F32, tag="bnrs")
                nc.vector.tensor_mul(out=rs[:, 1:2], in0=mv[:, 0:1],
                                     in1=mv[:, 0:1])
                nc.vector.tensor_add(out=rs[:, 1:2], in0=rs[:, 1:2],
                                     in1=mv[:, 1:2])
                nc.vector.tensor_copy(out=rs[:, 0:1], in_=mv[:, 0:1])
                rs_list.append(rs)

        if stop_after == "D":
            return
        # ---- phase E: gn2 combine + applies (stats computed above) ----
        hn_sb = [qkp.tile([128, T], BF16, tag="qk", name=f"hn{i}")
                 for i in range(KC)]
        with tc.tile_pool(name="gnps2", bufs=2, space="PSUM") as gnps:
            gp = gnps.tile([G, 2], F32, tag="gps")
            for k in range(KC):
                nc.tensor.matmul(gp, lhsT=sel_sb[:, k, :], rhs=rs_list[k],
                                 start=(k == 0), stop=(k == KC - 1))
            gg = stats.tile([128, 2], F32, tag="gng")
            nc.vector.memset(gg[32:64, :], 0.0)
            nc.vector.memset(gg[64:128, :], 0.0)
            nc.vector.tensor_copy(out=gg[:G, 0:1], in_=gp[:, 0:1])
            tmp = stats.tile([G, 1], F32, tag="gnt")
            nc.vector.tensor_mul(out=tmp, in0=gg[:G, 0:1], in1=gg[:G, 0:1])
            nc.vector.tensor_tensor(out=gg[:G, 1:2], in0=gp[:, 1:2], in1=tmp,
                                    op=ALU.subtract)
            nc.scalar.activation(out=gg[:G, 1:2], in_=gg[:G, 1:2], func=AF.Ln,
                                 bias=eps32, scale=1.0)
            nc.scalar.activation(out=gg[:G, 1:2], in_=gg[:G, 1:2], func=AF.Exp,
                                 scale=-0.5)
            for k in range(KC):
                ex = gnps.tile([128, 2], F32, tag="gex")
                nc.tensor.matmul(ex, lhsT=selt_sb[:, 128 * k:128 * (k + 1)],
                                 rhs=gg, start=True, stop=True)
                ab = stats.tile([128, 2], F32, tag="gnab")
                nc.vector.tensor_mul(out=ab[:, 0:1], in0=ex[:, 1:2],
                                     in1=gb_sb[:, 8 + k:8 + k + 1])
                nc.vector.tensor_mul(out=ab[:, 1:2], in0=ex[:, 0:1],
                                     in1=ab[:, 0:1])
                nc.vector.tensor_tensor(out=ab[:, 1:2],
                                        in0=gb_sb[:, 12 + k:13 + k],
                                        in1=ab[:, 1:2], op=ALU.subtract)
                if k < 2:
                    nc.scalar.activation(out=hn_sb[k][:, 0:T],
                                         in_=h_sb[k][:, 0:T],
                                         func=AF.Identity, bias=ab[:, 1:2],
                                         scale=ab[:, 0:1])
                elif k == 2:
                    nc.vector.tensor_scalar(out=hn_sb[k][:, 0:T],
                                            in0=h_sb[k][:, 0:T],
                                            scalar1=ab[:, 0:1],
                                            scalar2=ab[:, 1:2],
                                            op0=ALU.mult, op1=ALU.add)
                else:
                    nc.gpsimd.tensor_scalar(out=hn_sb[k][:, 0:T],
                                            in0=h_sb[k][:, 0:T],
                                            scalar1=ab[:, 0:1],
                                            scalar2=ab[:, 1:2],
                                            op0=ALU.mult, op1=ALU.add)

        if stop_after == "E":
            return
        # ---- phase F: proj + bias + residual (x kept resident in xt,
        # wp loaded at kernel start) ----
        xres = xt
        with tc.tile_pool(name="prps", bufs=3, space="PSUM") as prps:
            for mt in range(KC):
                for n in range(TC5):
                    ps = prps.tile([128, 512], F32, tag="pr")
                    for k in range(KC):
                        nc.tensor.matmul(
                            ps, lhsT=wp_sb[k][:, 128 * mt:128 * (mt + 1)],
                            rhs=hn_sb[k][:, 512 * n:512 * (n + 1)],
                            start=(k == 0), stop=(k == KC - 1))
                    ot = outp.tile([128, 512], F32, tag="ot")
                    nc.vector.scalar_tensor_tensor(
                        out=ot, in0=ps, scalar=bp_sb[:, mt:mt + 1],
                        in1=xres[mt][:, 512 * n:512 * (n + 1)],
                        op0=ALU.add, op1=ALU.add)
                    nc.sync.dma_start(
                        out_d[128 * mt:128 * (mt + 1), 512 * n:512 * (n + 1)], ot)


def _build_module(reps=1, stop_after="F"):
    nc = bacc.Bacc("TRN2", target_bir_lowering=False, debug=False,
                   num_devices=NCORES)
    d = {}

    def inp(name, shape, dt=F32):
        d[name] = nc.dram_tensor(name, shape, dt, kind="ExternalInput").ap()

    inp("x", [C, T])
    inp("wqk", [C, 1024], BF16)
    inp("bqk", [1024])
    inp("wv", [C, 512], BF16)
    inp("bv", [512])
    inp("wp", [C, C], BF16)
    inp("bp", [C])
    inp("g1", [C]); inp("b1", [C]); inp("g2", [C]); inp("b2", [C])
    inp("sel", [C, G])
    inp("selt", [128, C])
    out_d = nc.dram_tensor("out", [C, T], F32, kind="ExternalOutput").ap()

    with tile.TileContext(nc) as tc:
        _kernel_body(nc, tc, d, out_d, reps=reps, stop_after=stop_after)
    nc.compile()
    return nc


def _prep_weights(w_qkv, b_qkv, w_proj, b_proj):
    w_qkv = np.asarray(w_qkv, np.float32)
    b_qkv = np.asarray(b_qkv, np.float32)
    q = [w_qkv[192 * h:192 * h + 64] for h in range(H)]
    k = [w_qkv[192 * h + 64:192 * h + 128] for h in range(H)]
    v = [w_qkv[192 * h + 128:192 * h + 192] for h in range(H)]
    qb = [b_qkv[192 * h:192 * h + 64] for h in range(H)]
    kb = [b_qkv[192 * h + 64:192 * h + 128] for h in range(H)]
    vb = [b_qkv[192 * h + 128:192 * h + 192] for h in range(H)]

    wqk_rows = []
    bqk = []
    for p in range(H // 2):
        wqk_rows += [q[2 * p], q[2 * p + 1], k[2 * p], k[2 * p + 1]]
        bqk += [qb[2 * p], qb[2 * p + 1], kb[2 * p], kb[2 * p + 1]]
    wqk = np.ascontiguousarray(np.concatenate(wqk_rows, 0).T)      # [512,1024]
    bqk = np.concatenate(bqk, 0)                                   # [1024]

    wv = np.zeros((C, 512), np.float32)
    bv = np.zeros((512,), np.float32)
    for h in range(H):
        wv[:, 64 * h:64 * h + 64] = v[h].T
        bv[64 * h:64 * h + 64] = vb[h]

    wp = np.ascontiguousarray(np.asarray(w_proj, np.float32).T)
    bp = np.asarray(b_proj, np.float32)

    sel = np.zeros((C, G), np.float32)
    sel[np.arange(C), np.arange(C) // GS] = 1.0 / GS
    selt = np.zeros((128, C), np.float32)
    selt[np.arange(C) // GS, np.arange(C)] = 1.0

    bf = ml_dtypes.bfloat16
    return dict(wqk=wqk.astype(bf), bqk=bqk, wv=wv.astype(bf), bv=bv,
                wp=wp.astype(bf), bp=bp, sel=sel, selt=selt)


def _make_in_maps(x, gn1_gamma, gn1_beta, w_qkv, b_qkv, gn2_gamma, gn2_beta,
                  w_proj, b_proj):
    x = np.asarray(x, np.float32)
    shared = _prep_weights(w_qkv, b_qkv, w_proj, b_proj)
    shared.update(g1=np.asarray(gn1_gamma, np.float32),
                  b1=np.asarray(gn1_beta, np.float32),
                  g2=np.asarray(gn2_gamma, np.float32),
                  b2=np.asarray(gn2_beta, np.float32))
    return [dict(shared, x=np.ascontiguousarray(x[c])) for c in range(NCORES)]


def kernel(x, gn1_gamma, gn1_beta, w_qkv, b_qkv, gn2_gamma, gn2_beta, w_proj,
           b_proj):
    if "nc" not in _CACHE:
        _CACHE["nc"] = _build_module(reps=1)
    nc = _CACHE["nc"]
    in_maps = _make_in_maps(x, gn1_gamma, gn1_beta, w_qkv, b_qkv, gn2_gamma,
                            gn2_beta, w_proj, b_proj)
    res = run_bass_kernel_spmd(nc, in_maps, core_ids=list(range(NCORES)))
    out = np.stack([res.results[c]["out"] for c in range(NCORES)], 0)
    return out.astype(np.float32)


def _make_runner(nc, in_maps):
    """Cached jitted executor with device-resident inputs; per-call cost is
    dispatch + device execution only (no host transfers, no retrace)."""
    import jax
    import jax.numpy as jnp
    from jax.experimental.shard_map import shard_map
    from jax.sharding import Mesh, PartitionSpec, NamedSharding
    from concourse import bass2jax, mybir as mb

    bass2jax.install_neuronx_cc_hook()
    part_name = nc.partition_id_tensor.name if nc.partition_id_tensor else None
    in_names, out_names, out_avals, zero_outs = [], [], [], []
    for alloc in nc.m.functions[0].allocations:
        if not isinstance(alloc, mb.MemoryLocationSet):
            continue
        name = alloc.memorylocations[0].name
        if alloc.kind == "ExternalInput":
            if name != part_name:
                in_names.append(name)
        elif alloc.kind == "ExternalOutput":
            out_names.append(name)
            shape = tuple(alloc.tensor_shape)
            dtype = mb.dt.np(alloc.dtype)
            out_avals.append(jax.core.ShapedArray(shape, dtype))
            zero_outs.append(np.zeros(shape, dtype))
    n_params = len(in_names)
    all_names = in_names + out_names + ([part_name] if part_name else [])

    def _body(*args):
        operands = list(args)
        if part_name:
            operands.append(bass2jax.partition_id_tensor())
        outs = bass2jax._bass_exec_p.bind(
            *operands, out_avals=tuple(out_avals), in_names=tuple(all_names),
            out_names=tuple(out_names), lowering_input_output_aliases=(),
            sim_require_finite=True, sim_require_nnan=True, nc=nc)
        return tuple(outs)

    devices = jax.devices()[:NCORES]
    mesh = Mesh(np.asarray(devices), ("core",))
    spec = PartitionSpec("core")
    fn = jax.jit(shard_map(_body, mesh=mesh,
                           in_specs=(spec,) * (n_params + len(out_names)),
                           out_specs=(spec,) * len(out_names),
                           check_rep=False), keep_unused=True)
    sh = NamedSharding(mesh, spec)
    dev_args = [
        jax.device_put(
            np.concatenate([np.asarray(in_maps[c][nm])[None] for c in
                            range(NCORES)], 0).reshape(
                NCORES * np.asarray(in_maps[0][nm]).shape[0],
                *np.asarray(in_maps[0][nm]).shape[1:]), sh)
        for nm in in_names
    ] + [
        jax.device_put(np.zeros((NCORES * z.shape[0], *z.shape[1:]), z.dtype),
                       sh) for z in zero_outs
    ]

    def call():
        outs = fn(*dev_args)
        jax.block_until_ready(outs)
        return outs

    return call


def bench(inputs, rep_list=(33, 65), n_calls=9, stop_after="F"):
    """Estimate on-device kernel time by the slope method: per-call wall time
    of an R-rep hardware loop for two R values; the difference cancels
    dispatch overhead.  Calls for the two rep counts are interleaved because
    wall times are bimodal (a constant ~50ms dispatch artifact appears on
    some calls): interleaving gives both configs a fair shot at the fast
    state, and min() selects it."""
    import time
    in_maps = _make_in_maps(**inputs)
    runners = {}
    for reps in rep_list:
        key = f"nc{reps}_{stop_after}"
        if key not in _CACHE:
            _CACHE[key] = _build_module(reps=reps, stop_after=stop_after)
        runners[reps] = _make_runner(_CACHE[key], in_maps)
        runners[reps]()  # warmup (compile+load)
    times = {reps: [] for reps in rep_list}
    for i in range(n_calls):
        if i % 3 == 2:
            time.sleep(0.3)  # decorrelate the bimodal dispatch state
        for reps in rep_list:
            t0 = time.time()
            runners[reps]()
            times[reps].append(time.time() - t0)
    import statistics
    lo, hi = min(rep_list), max(rep_list)
    mins, meds, fast = {}, {}, {}
    for reps in rep_list:
        mins[reps] = min(times[reps])
        meds[reps] = statistics.median(times[reps])
        # the per-call dispatch overhead is bimodal (a constant ~45ms state
        # appears on most calls); a min is a "fast-state" sample when it sits
        # well below the median
        fast[reps] = (meds[reps] - mins[reps]) > 0.025
        print(f"reps={reps}: call walls "
              f"{[f'{t*1e3:.1f}ms' for t in times[reps]]}", flush=True)
    if fast[lo] == fast[hi]:
        # both mins from the same dispatch state: difference is clean
        est = (mins[hi] - mins[lo]) / (hi - lo)
    else:
        # states differ: difference the slow-state floors (min over samples
        # near or above the median), which both configs sample reliably
        slow = {r: min(t for t in times[r] if t > meds[r] - 0.025)
                for r in rep_list}
        est = (slow[hi] - slow[lo]) / (hi - lo)
        print("(min states differ: using slow-state floor slope)", flush=True)
    print(f"estimated per-iteration kernel time: {est * 1e9:.0f} ns")
    return est
